# revision 37
# baseline (speedup 1.0000x reference)
"""BayesianNN (attention over memory + 2-pass genome gemv) on 8 Trainium2 cores.

Strategy (memory-bound; QKV weights dominate the wire bytes):
  * Column-shard (tensor-parallel) the three QKV projection matrices across
    the 8 cores. The host pre-transposes, pre-casts to fp16 and pre-tiles
    each shard as [128, 60, 961] (+ a [7, 961] tail holding rows 7680:7686
    and the folded bias row), so the device streams exactly the needed
    bytes (~45 MB/core) through plain HWDGE DMAs in ~1.4 MB chunks and
    matmuls against a resident fp16 x^T with f32 PSUM accumulation.
  * The genome sample W = W_mu + W_sigma*eps_w is only ever needed at
    columns [D:N]; the host samples those slices directly (g: per-core
    row shard [961, 130] fp16, h: [130, 2], b: [130]).
  * Stream order v -> k -> q, with the weight chunks alternating between
    the two HWDGE rings (sync/scalar). The Y = v^T @ g partial and its
    AllReduce ride hidden under the k/q streaming (also absorbing the
    ~11.5us first-collective delay and mid-stream rank skew); only a
    scores-only fp16 AllGather (32 KB, summed on-chip) sits on the tail.
    Everything downstream is a few tiny matmuls + a fused softmax,
    identical on every core.
"""

import numpy as np

D = 7686
M = 128
NH = 128
NO = 2
N = D + NH + NO          # 7816
NCORES = 8
JW = 961                 # per-core shard width (cols of q/k/v); core 7 pads 959->961
NIT = 61                 # i-tiles over the contraction (60 full + 7-row tail)
NROW = 60 * 128 + 7      # 7687 = D rows + 1 bias row
CHUNK = 4                # i-tiles per weight DMA (4*961*2 B/partition)
GCH = [128] * 7 + [65]   # j-chunks of the 961-wide shard for transposes/Y/scores
SQRT_D = float(np.sqrt(np.float32(D)))

_COMPILED = None


def _build_program():
    import concourse.bacc as bacc
    import concourse.tile as tile
    import concourse.mybir as mybir
    from concourse import masks

    f32, f16 = mybir.dt.float32, mybir.dt.float16
    AF = mybir.ActivationFunctionType

    nc = bacc.Bacc("TRN2", debug=False, num_devices=NCORES)

    wT = {m: nc.dram_tensor(f"{m}T", [128, 60, JW], f16, kind="ExternalInput").ap()
          for m in ("v", "k", "q")}
    wTt = {m: nc.dram_tensor(f"{m}Tt", [7, JW], f16, kind="ExternalInput").ap()
           for m in ("v", "k", "q")}
    xT_d = nc.dram_tensor("xT", [128, 60, M], f16, kind="ExternalInput").ap()
    xTt_d = nc.dram_tensor("xTt", [7, M], f16, kind="ExternalInput").ap()
    g_d = nc.dram_tensor("g", [128, 8, NH + NO], f16, kind="ExternalInput").ap()
    h_d = nc.dram_tensor("h", [NH + NO, NO], f32, kind="ExternalInput").ap()
    b_d = nc.dram_tensor("b", [NH + NO], f32, kind="ExternalInput").ap()
    out_d = nc.dram_tensor("out", [NO], f32, kind="ExternalOutput").ap()

    with tile.TileContext(nc) as tc:
        with (
            tc.tile_pool(name="const", bufs=1) as constp,
            tc.tile_pool(name="stream", bufs=10) as streamp,
            tc.tile_pool(name="streamt", bufs=2) as streamtp,
            tc.tile_pool(name="big", bufs=1) as bigp,
            tc.tile_pool(name="small", bufs=2) as smallp,
            tc.tile_pool(name="ps_stream", bufs=2, space="PSUM") as ps_stream,
            tc.tile_pool(name="ps_small", bufs=2, space="PSUM") as ps_small,
            tc.tile_pool(name="ps_misc", bufs=1, space="PSUM") as ps_misc,
            tc.tile_pool(name="ps_phi", bufs=1, space="PSUM") as ps_phi,
            tc.tile_pool(name="dram", bufs=1, space="DRAM") as dramp,
        ):
            # ---- resident constants -------------------------------------
            ident = constp.tile([128, 128], f16)
            masks.make_identity(nc, ident[:])
            warm = constp.tile([128, 128], f16)
            nc.vector.memset(warm[:], 0.0)

            # x^T resident in SBUF as fp16, one i-tile per 128-col block.
            # First 8 i-tiles land first so the v matmuls can start early.
            xT_sb = constp.tile([128, NIT * M], f16)
            xs3 = xT_sb[:].rearrange("p (t m) -> p t m", m=M)
            nc.sync.dma_start(xs3[:, 0:8, :], xT_d[:, 0:8, :])

            # genome (host-sampled): g = W[:D,D:N] row-shard, h = W[D:N,N-2:N],
            # b = bias[D:N]
            g_sb = constp.tile([128, 8 * (NH + NO)], f16)
            g3 = g_sb[:].rearrange("p (c t) -> p c t", t=NH + NO)
            nc.scalar.dma_start(g3[:, :, :], g_d[:, :, :])
            h_lo = constp.tile([128, NO], f32)
            h_hi = constp.tile([NO, NO], f32)
            nc.scalar.dma_start(h_lo[:], h_d[0:NH, :])
            nc.scalar.dma_start(h_hi[:], h_d[NH:NH + NO, :])
            b_lo = constp.tile([128, 1], f32)
            b_hi = constp.tile([NO, 1], f32)
            nc.scalar.dma_start(b_lo[:], b_d[0:NH])
            nc.scalar.dma_start(b_hi[:], b_d[NH:NH + NO])

            # Two collectives: the Y AllReduce fires right after the v stream
            # (hidden under k/q streaming; it also absorbs the deterministic
            # ~11.5us first-collective delay and resyncs rank skew), and a
            # scores-only fp16 AllGather (32 KB) at the end with the 8-way sum
            # done on-chip.
            ag_in = dramp.tile([M, M], f16)
            ag_out = dramp.tile([NCORES * M, M], f16)
            y_in = dramp.tile([M, NH + NO], f32)
            y_out = dramp.tile([M, NH + NO], f32)
            groups = [list(range(NCORES))]

            # PE warm-up on the memset tile while the first DMAs land
            psw = ps_misc.tile([128, NH + NO], f32, tag="gen", name="psw")
            for r in range(16):
                nc.tensor.matmul(psw[:, 0:128], warm[:], warm[:],
                                 start=True, stop=True, skip_group_check=True)

            # ---- QKV streaming ------------------------------------------
            qkvT_sb = {}     # j-partitioned [j, m] fp16 tiles per matrix

            def stream_mat(mat, chunk_hook=None):
                ps_a = ps_stream.tile([128, 512], f32, tag="wa", name=f"psa_{mat}")
                ps_b = ps_stream.tile([128, JW - 512], f32, tag="wb", name=f"psb_{mat}")
                it = 0
                for ci, c0 in enumerate(range(0, 60, CHUNK)):
                    if chunk_hook is not None:
                        chunk_hook(ci)
                    wt = streamp.tile([128, CHUNK * JW], f16, tag="wt",
                                      name=f"wt_{mat}_{c0}")
                    eng = nc.sync if ci % 2 == 0 else nc.scalar
                    eng.dma_start(
                        wt[:].rearrange("p (c j) -> p c j", j=JW),
                        wT[mat][:, c0:c0 + CHUNK, :])
                    for k in range(CHUNK):
                        lhsT = xT_sb[:, it * M:(it + 1) * M]
                        rhs = wt[:, k * JW:(k + 1) * JW]
                        nc.tensor.matmul(ps_a[:], lhsT, rhs[:, 0:512],
                                         start=(it == 0), stop=False)
                        nc.tensor.matmul(ps_b[:], lhsT, rhs[:, 512:JW],
                                         start=(it == 0), stop=False)
                        it += 1
                # 7-row tail tile (holds the bias row)
                wtt = streamtp.tile([128, JW], f16, tag="wtt", name=f"wtt_{mat}")
                nc.sync.dma_start(wtt[0:7, :], wTt[mat][:, :])
                lhsT = xT_sb[0:7, 60 * M:61 * M]
                nc.tensor.matmul(ps_a[:], lhsT, wtt[0:7, 0:512],
                                 start=False, stop=True)
                nc.tensor.matmul(ps_b[:], lhsT, wtt[0:7, 512:JW],
                                 start=False, stop=True)

                # PSUM -> SBUF (cast to fp16), then PE-transpose to [j, m]
                sb = bigp.tile([128, JW], f16, tag=f"{mat}_sb", name=f"{mat}_sb")
                sbT = bigp.tile([128, 8 * 128], f16, tag=f"{mat}T", name=f"{mat}T")
                for jt, jw in enumerate(GCH):
                    j0 = jt * 128
                    if j0 + jw <= 512:
                        nc.vector.tensor_copy(sb[:, j0:j0 + jw],
                                              ps_a[:, j0:j0 + jw])
                    else:
                        nc.vector.tensor_copy(sb[:, j0:j0 + jw],
                                              ps_b[:, j0 - 512:j0 - 512 + jw])
                    psT = ps_small.tile([128, 128], f16, tag="psT",
                                        name=f"psT_{mat}{jt}")
                    nc.tensor.transpose(psT[:jw, :], sb[:, j0:j0 + jw], ident[:])
                    nc.vector.tensor_copy(sbT[:jw, jt * 128:(jt + 1) * 128],
                                          psT[:jw, :])
                qkvT_sb[mat] = sbT

            def xt_rest(ci):
                if ci == 1:
                    nc.sync.dma_start(xs3[:, 8:60, :], xT_d[:, 8:60, :])
                elif ci == 2:
                    nc.sync.dma_start(xs3[0:7, 60, :], xTt_d[:, :])

            stream_mat("v", chunk_hook=xt_rest)

            # Y = v_shard^T @ g  (attention-independent, shard-summable);
            # its AllReduce hides under the k/q streams.
            ps_y = ps_misc.tile([128, NH + NO], f32, tag="gen", name="ps_y")
            for ch, chw in enumerate(GCH):
                nc.tensor.matmul(
                    ps_y[:], qkvT_sb["v"][:chw, ch * 128:ch * 128 + 128],
                    g_sb[:chw, ch * (NH + NO):(ch + 1) * (NH + NO)],
                    start=(ch == 0), stop=(ch == 7))
            y_sb = smallp.tile([128, NH + NO], f32)
            nc.vector.tensor_copy(y_sb[:], ps_y[:])
            nc.gpsimd.dma_start(y_in[:], y_sb[:])
            nc.gpsimd.collective_compute(
                "AllReduce", mybir.AluOpType.add, replica_groups=groups,
                ins=[y_in.opt()], outs=[y_out.opt()])
            yf = smallp.tile([128, NH + NO], f32)
            nc.scalar.dma_start(yf[:], y_out[:])

            stream_mat("k")
            stream_mat("q")

            # partial scores over the local j-shard
            ps_s = ps_misc.tile([128, 128], f32, tag="gen", name="ps_s")
            for jt, jw in enumerate(GCH):
                nc.tensor.matmul(
                    ps_s[:],
                    qkvT_sb["q"][:jw, jt * 128:jt * 128 + 128],
                    qkvT_sb["k"][:jw, jt * 128:jt * 128 + 128],
                    start=(jt == 0), stop=(jt == 7))
            sc_sb = smallp.tile([128, 128], f16)
            nc.vector.tensor_copy(sc_sb[:], ps_s[:])
            nc.gpsimd.dma_start(ag_in[:], sc_sb[:])
            nc.gpsimd.collective_compute(
                "AllGather", mybir.AluOpType.bypass, replica_groups=groups,
                ins=[ag_in.opt()], outs=[ag_out.opt()])

            # read all 8 score partials back and sum on-chip
            agf = smallp.tile([128, NCORES * M], f16)
            nc.scalar.dma_start(
                agf[:].rearrange("p (r c) -> p r c", c=M),
                ag_out.rearrange("(r p) c -> p r c", p=M))
            acc = smallp.tile([128, M], f16)
            nc.vector.tensor_add(acc[:], agf[:, 0:M], agf[:, M:2 * M])
            for r in range(2, NCORES):
                nc.vector.tensor_add(acc[:], acc[:], agf[:, r * M:(r + 1) * M])

            # softmax over the free axis of scores/sqrt(D); logits are ~N(0,1)
            # so no max-subtraction is needed. Exp's accum_out gives the row
            # sums in the same instruction; 1/M folds into the reciprocal.
            att = smallp.tile([128, 128], f32)
            ssum = smallp.tile([128, 1], f32)
            nc.scalar.activation(att[:], acc[:], AF.Exp, scale=1.0 / SQRT_D,
                                 accum_out=ssum[:])
            rinv = smallp.tile([128, 1], f32)
            nc.vector.reciprocal(rinv[:], ssum[:])

            # w[m'] = sum_m attn[m, m'] / rowsum[m]; the 1/M mean folds into
            # the PSUM->SBUF copy's scale
            ps_w = ps_misc.tile([128, 1], f32, tag="gen", name="ps_w")
            nc.tensor.matmul(ps_w[:], att[:], rinv[:])
            w_sb = smallp.tile([128, 1], f32)
            nc.scalar.activation(w_sb[:], ps_w[:], AF.Copy, scale=1.0 / M)

            pre_lo = ps_misc.tile([128, 1], f32, tag="gen", name="pre_lo")
            nc.tensor.matmul(pre_lo[:], yf[:, 0:NH], w_sb[:])
            pre_hi = ps_phi.tile([NO, 1], f32, tag="phi", name="pre_hi")
            nc.tensor.matmul(pre_hi[:], yf[:, NH:NH + NO], w_sb[:])

            # h = tanh(pre1 + b1); fin = tanh(pre1_hi + b2 + h @ W2)
            hl = smallp.tile([128, 1], f32)
            nc.scalar.activation(hl[:], pre_lo[:], AF.Tanh, bias=b_lo[:])
            phb = smallp.tile([NO, 1], f32)
            nc.vector.tensor_add(phb[:], pre_hi[:], b_hi[:])
            hh = smallp.tile([NO, 1], f32)
            nc.scalar.activation(hh[:], phb[:], AF.Tanh)

            ps_f = ps_misc.tile([NO, 1], f32, tag="gen", name="ps_f")
            nc.tensor.matmul(ps_f[:], h_lo[:NH, :], hl[:], start=True, stop=False)
            nc.tensor.matmul(ps_f[:], h_hi[:NO, :], hh[:], start=False, stop=True)
            fin = smallp.tile([NO, 1], f32)
            nc.scalar.activation(fin[:], ps_f[:], AF.Tanh, bias=phb[:])
            nc.scalar.dma_start(out_d[:], fin[:])

    nc.compile()
    return nc


def _shard_inputs(inputs):
    x = np.ascontiguousarray(inputs["x"], dtype=np.float32)
    xTf = np.zeros((NROW, M), np.float32)
    xTf[:D, :] = x.T
    xTf[D, :] = 1.0                     # bias row
    xT_body = np.ascontiguousarray(
        xTf[:60 * 128].reshape(60, 128, M).transpose(1, 0, 2)).astype(np.float16)
    xT_tail = xTf[60 * 128:].astype(np.float16)

    widths = [min(961, D - 961 * c) for c in range(NCORES)]
    offs = [961 * c for c in range(NCORES)]

    # host-side Bayesian sample, only the slices the graph ever reads
    Ws_cols = (inputs["W_mu"][:, D:N] + inputs["W_sigma"][:, D:N]
               * inputs["eps_w"][:, D:N]).astype(np.float32)       # [N, 130]
    h_full = np.ascontiguousarray(
        Ws_cols[D:N, NH:NH + NO], dtype=np.float32)                # [130, 2]
    b_full = (inputs["bias_mu"][D:N] + inputs["bias_sigma"][D:N]
              * inputs["eps_b"][D:N]).astype(np.float32)           # [130]

    in_maps = []
    for c in range(NCORES):
        off, w = offs[c], widths[c]
        im = {"xT": xT_body, "xTt": xT_tail, "h": h_full, "b": b_full}
        for mat, Wn, bn in (("q", "Wq", "bq"), ("k", "Wk", "bk"), ("v", "Wv", "bv")):
            Wt = np.zeros((NROW, JW), np.float32)
            Wt[:D, :w] = inputs[Wn][off:off + w, :].T
            Wt[D, :w] = inputs[bn][off:off + w]
            im[f"{mat}T"] = np.ascontiguousarray(
                Wt[:60 * 128].reshape(60, 128, JW).transpose(1, 0, 2)
            ).astype(np.float16)
            im[f"{mat}Tt"] = Wt[60 * 128:].astype(np.float16)
        g = np.zeros((8 * 128, NH + NO), np.float32)
        g[:w, :] = Ws_cols[off:off + w, :]
        im["g"] = np.ascontiguousarray(
            g.reshape(8, 128, NH + NO).transpose(1, 0, 2)).astype(np.float16)
        in_maps.append(im)
    return in_maps


def _run(inputs, trace=False):
    global _COMPILED
    from concourse.bass_utils import run_bass_kernel_spmd

    if _COMPILED is None:
        _COMPILED = _build_program()
    in_maps = _shard_inputs(inputs)
    res = run_bass_kernel_spmd(
        _COMPILED, in_maps, core_ids=list(range(NCORES)), trace=trace)
    out = np.asarray(res.results[0]["out"], dtype=np.float32).reshape(NO)
    return out, res


def kernel(**inputs):
    out, _ = _run(inputs, trace=False)
    return out


# revision 38
# speedup vs baseline: 1.3975x; 1.3975x over previous
"""BayesianNN (attention over memory + 2-pass genome gemv) on 8 Trainium2 cores.

Strategy (memory-bound; QKV weights dominate the wire bytes):
  * Column-shard (tensor-parallel) the three QKV projection matrices across
    the 8 cores. The host pre-transposes, pre-casts to fp16 and pre-tiles
    each shard as [128, 60, 961] (+ a [7, 961] tail holding rows 7680:7686
    and the folded bias row), so the device streams exactly the needed
    bytes (~45 MB/core) through plain HWDGE DMAs in ~1.4 MB chunks and
    matmuls against a resident fp16 x^T with f32 PSUM accumulation.
  * The genome sample W = W_mu + W_sigma*eps_w is only ever needed at
    columns [D:N]; the host samples those slices directly (g: per-core
    row shard [961, 130] fp16, h: [130, 2], b: [130]).
  * Stream order v -> k -> q, with the weight chunks alternating between
    the two HWDGE rings (sync/scalar). The Y = v^T @ g partial and its
    AllReduce ride hidden under the k/q streaming (also absorbing the
    ~11.5us first-collective delay and mid-stream rank skew); only a
    scores-only fp16 AllGather (32 KB, summed on-chip) sits on the tail.
    Everything downstream is a few tiny matmuls + a fused softmax,
    identical on every core.
"""

import numpy as np

D = 7686
M = 128
NH = 128
NO = 2
N = D + NH + NO          # 7816
NCORES = 8
JW = 961                 # per-core shard width (cols of q/k/v); core 7 pads 959->961
NIT = 61                 # i-tiles over the contraction (60 full + 7-row tail)
NROW = 60 * 128 + 7      # 7687 = D rows + 1 bias row
CHUNK = 4                # i-tiles per weight DMA (4*961*2 B/partition)
GCH = [128] * 7 + [65]   # j-chunks of the 961-wide shard for transposes/Y/scores
SQRT_D = float(np.sqrt(np.float32(D)))

_COMPILED = None


def _build_program():
    import concourse.bacc as bacc
    import concourse.tile as tile
    import concourse.mybir as mybir
    from concourse import masks

    f32, f16 = mybir.dt.float32, mybir.dt.float16
    AF = mybir.ActivationFunctionType

    nc = bacc.Bacc("TRN2", debug=False, num_devices=NCORES)

    wT = {m: nc.dram_tensor(f"{m}T", [128, 60, JW], f16, kind="ExternalInput").ap()
          for m in ("v", "k", "q")}
    wTt = {m: nc.dram_tensor(f"{m}Tt", [7, JW], f16, kind="ExternalInput").ap()
           for m in ("v", "k", "q")}
    xT_d = nc.dram_tensor("xT", [128, 60, M], f16, kind="ExternalInput").ap()
    xTt_d = nc.dram_tensor("xTt", [7, M], f16, kind="ExternalInput").ap()
    g_d = nc.dram_tensor("g", [128, 8, NH + NO], f16, kind="ExternalInput").ap()
    h_d = nc.dram_tensor("h", [NH + NO, NO], f32, kind="ExternalInput").ap()
    b_d = nc.dram_tensor("b", [NH + NO], f32, kind="ExternalInput").ap()
    out_d = nc.dram_tensor("out", [NO], f32, kind="ExternalOutput").ap()

    with tile.TileContext(nc) as tc:
        with (
            tc.tile_pool(name="const", bufs=1) as constp,
            tc.tile_pool(name="stream", bufs=10) as streamp,
            tc.tile_pool(name="streamt", bufs=2) as streamtp,
            tc.tile_pool(name="big", bufs=1) as bigp,
            tc.tile_pool(name="small", bufs=2) as smallp,
            tc.tile_pool(name="ps_stream", bufs=2, space="PSUM") as ps_stream,
            tc.tile_pool(name="ps_small", bufs=2, space="PSUM") as ps_small,
            tc.tile_pool(name="ps_misc", bufs=1, space="PSUM") as ps_misc,
            tc.tile_pool(name="ps_phi", bufs=1, space="PSUM") as ps_phi,
            tc.tile_pool(name="dram", bufs=1, space="DRAM") as dramp,
        ):
            # ---- resident constants -------------------------------------
            ident = constp.tile([128, 128], f16)
            masks.make_identity(nc, ident[:])
            warm = constp.tile([128, 128], f16)
            nc.vector.memset(warm[:], 0.0)

            # x^T resident in SBUF as fp16, one i-tile per 128-col block.
            # First 8 i-tiles land first so the v matmuls can start early.
            xT_sb = constp.tile([128, NIT * M], f16)
            xs3 = xT_sb[:].rearrange("p (t m) -> p t m", m=M)
            nc.gpsimd.dma_start(xs3[:, 0:8, :], xT_d[:, 0:8, :])

            # genome (host-sampled): g = W[:D,D:N] row-shard, h = W[D:N,N-2:N],
            # b = bias[D:N]
            g_sb = constp.tile([128, 8 * (NH + NO)], f16)
            g3 = g_sb[:].rearrange("p (c t) -> p c t", t=NH + NO)
            nc.gpsimd.dma_start(g3[:, :, :], g_d[:, :, :])
            h_lo = constp.tile([128, NO], f32)
            h_hi = constp.tile([NO, NO], f32)
            nc.gpsimd.dma_start(h_lo[:], h_d[0:NH, :])
            nc.gpsimd.dma_start(h_hi[:], h_d[NH:NH + NO, :])
            b_lo = constp.tile([128, 1], f32)
            b_hi = constp.tile([NO, 1], f32)
            nc.gpsimd.dma_start(b_lo[:], b_d[0:NH])
            nc.gpsimd.dma_start(b_hi[:], b_d[NH:NH + NO])

            # Two collectives: the Y AllReduce fires right after the v stream
            # (hidden under k/q streaming; it also absorbs the deterministic
            # ~11.5us first-collective delay and resyncs rank skew), and a
            # scores-only fp16 AllGather (32 KB) at the end with the 8-way sum
            # done on-chip.
            ag_in = dramp.tile([M, M], f16)
            ag_out = dramp.tile([NCORES * M, M], f16)
            y_in = dramp.tile([M, NH + NO], f32)
            y_out = dramp.tile([M, NH + NO], f32)
            groups = [list(range(NCORES))]

            # PE warm-up on the memset tile while the first DMAs land
            psw = ps_misc.tile([128, NH + NO], f32, tag="gen", name="psw")
            for r in range(16):
                nc.tensor.matmul(psw[:, 0:128], warm[:], warm[:],
                                 start=True, stop=True, skip_group_check=True)

            # ---- QKV streaming ------------------------------------------
            qkvT_sb = {}     # j-partitioned [j, m] fp16 tiles per matrix

            def stream_mat(mat, chunk_hook=None):
                ps_a = ps_stream.tile([128, 512], f32, tag="wa", name=f"psa_{mat}")
                ps_b = ps_stream.tile([128, JW - 512], f32, tag="wb", name=f"psb_{mat}")
                it = 0
                plan = [2, 2] + [CHUNK] * 14 if mat == "v" else [CHUNK] * 15
                c0 = 0
                for ci, cw in enumerate(plan):
                    if chunk_hook is not None:
                        chunk_hook(ci)
                    wt = streamp.tile([128, CHUNK * JW], f16, tag="wt",
                                      name=f"wt_{mat}_{c0}")
                    eng = nc.sync if ci % 2 == 0 else nc.scalar
                    eng.dma_start(
                        wt[:, 0:cw * JW].rearrange("p (c j) -> p c j", j=JW),
                        wT[mat][:, c0:c0 + cw, :])
                    c0 += cw
                    for k in range(cw):
                        lhsT = xT_sb[:, it * M:(it + 1) * M]
                        rhs = wt[:, k * JW:(k + 1) * JW]
                        nc.tensor.matmul(ps_a[:], lhsT, rhs[:, 0:512],
                                         start=(it == 0), stop=False)
                        nc.tensor.matmul(ps_b[:], lhsT, rhs[:, 512:JW],
                                         start=(it == 0), stop=False)
                        it += 1
                # 7-row tail tile (holds the bias row)
                wtt = streamtp.tile([128, JW], f16, tag="wtt", name=f"wtt_{mat}")
                nc.sync.dma_start(wtt[0:7, :], wTt[mat][:, :])
                lhsT = xT_sb[0:7, 60 * M:61 * M]
                nc.tensor.matmul(ps_a[:], lhsT, wtt[0:7, 0:512],
                                 start=False, stop=True)
                nc.tensor.matmul(ps_b[:], lhsT, wtt[0:7, 512:JW],
                                 start=False, stop=True)

                # PSUM -> SBUF (cast to fp16), then PE-transpose to [j, m]
                sb = bigp.tile([128, JW], f16, tag=f"{mat}_sb", name=f"{mat}_sb")
                sbT = bigp.tile([128, 8 * 128], f16, tag=f"{mat}T", name=f"{mat}T")
                for jt, jw in enumerate(GCH):
                    j0 = jt * 128
                    if j0 + jw <= 512:
                        nc.vector.tensor_copy(sb[:, j0:j0 + jw],
                                              ps_a[:, j0:j0 + jw])
                    else:
                        nc.vector.tensor_copy(sb[:, j0:j0 + jw],
                                              ps_b[:, j0 - 512:j0 - 512 + jw])
                    psT = ps_small.tile([128, 128], f16, tag="psT",
                                        name=f"psT_{mat}{jt}")
                    nc.tensor.transpose(psT[:jw, :], sb[:, j0:j0 + jw], ident[:])
                    nc.vector.tensor_copy(sbT[:jw, jt * 128:(jt + 1) * 128],
                                          psT[:jw, :])
                qkvT_sb[mat] = sbT

            def xt_rest(ci):
                if ci == 1:
                    nc.sync.dma_start(xs3[:, 8:60, :], xT_d[:, 8:60, :])
                elif ci == 2:
                    nc.sync.dma_start(xs3[0:7, 60, :], xTt_d[:, :])

            stream_mat("v", chunk_hook=xt_rest)

            # Y = v_shard^T @ g  (attention-independent, shard-summable);
            # its AllReduce hides under the k/q streams.
            ps_y = ps_misc.tile([128, NH + NO], f32, tag="gen", name="ps_y")
            for ch, chw in enumerate(GCH):
                nc.tensor.matmul(
                    ps_y[:], qkvT_sb["v"][:chw, ch * 128:ch * 128 + 128],
                    g_sb[:chw, ch * (NH + NO):(ch + 1) * (NH + NO)],
                    start=(ch == 0), stop=(ch == 7))
            y_sb = smallp.tile([128, NH + NO], f32)
            nc.vector.tensor_copy(y_sb[:], ps_y[:])
            nc.gpsimd.dma_start(y_in[:], y_sb[:])
            nc.gpsimd.collective_compute(
                "AllReduce", mybir.AluOpType.add, replica_groups=groups,
                ins=[y_in.opt()], outs=[y_out.opt()])
            yf = smallp.tile([128, NH + NO], f32)
            nc.scalar.dma_start(yf[:], y_out[:])

            stream_mat("k")
            stream_mat("q")

            # partial scores over the local j-shard
            ps_s = ps_misc.tile([128, 128], f32, tag="gen", name="ps_s")
            for jt, jw in enumerate(GCH):
                nc.tensor.matmul(
                    ps_s[:],
                    qkvT_sb["q"][:jw, jt * 128:jt * 128 + 128],
                    qkvT_sb["k"][:jw, jt * 128:jt * 128 + 128],
                    start=(jt == 0), stop=(jt == 7))
            sc_sb = smallp.tile([128, 128], f16)
            nc.vector.tensor_copy(sc_sb[:], ps_s[:])
            nc.sync.dma_start(ag_in[:], sc_sb[:])
            nc.gpsimd.collective_compute(
                "AllGather", mybir.AluOpType.bypass, replica_groups=groups,
                ins=[ag_in.opt()], outs=[ag_out.opt()])

            # read all 8 score partials back and sum on-chip
            agf = smallp.tile([128, NCORES * M], f16)
            nc.sync.dma_start(
                agf[:].rearrange("p (r c) -> p r c", c=M),
                ag_out.rearrange("(r p) c -> p r c", p=M))
            acc = smallp.tile([128, M], f16)
            nc.vector.tensor_add(acc[:], agf[:, 0:M], agf[:, M:2 * M])
            for r in range(2, NCORES):
                nc.vector.tensor_add(acc[:], acc[:], agf[:, r * M:(r + 1) * M])

            # softmax over the free axis of scores/sqrt(D); logits are ~N(0,1)
            # so no max-subtraction is needed. Exp's accum_out gives the row
            # sums in the same instruction; 1/M folds into the reciprocal.
            att = smallp.tile([128, 128], f32)
            ssum = smallp.tile([128, 1], f32)
            nc.scalar.activation(att[:], acc[:], AF.Exp, scale=1.0 / SQRT_D,
                                 accum_out=ssum[:])
            rinv = smallp.tile([128, 1], f32)
            nc.vector.reciprocal(rinv[:], ssum[:])

            # w[m'] = sum_m attn[m, m'] / rowsum[m]; the 1/M mean folds into
            # the PSUM->SBUF copy's scale
            ps_w = ps_misc.tile([128, 1], f32, tag="gen", name="ps_w")
            nc.tensor.matmul(ps_w[:], att[:], rinv[:])
            w_sb = smallp.tile([128, 1], f32)
            nc.scalar.activation(w_sb[:], ps_w[:], AF.Copy, scale=1.0 / M)

            pre_lo = ps_misc.tile([128, 1], f32, tag="gen", name="pre_lo")
            nc.tensor.matmul(pre_lo[:], yf[:, 0:NH], w_sb[:])
            pre_hi = ps_phi.tile([NO, 1], f32, tag="phi", name="pre_hi")
            nc.tensor.matmul(pre_hi[:], yf[:, NH:NH + NO], w_sb[:])

            # h = tanh(pre1 + b1); fin = tanh(pre1_hi + b2 + h @ W2)
            hl = smallp.tile([128, 1], f32)
            nc.scalar.activation(hl[:], pre_lo[:], AF.Tanh, bias=b_lo[:])
            phb = smallp.tile([NO, 1], f32)
            nc.vector.tensor_add(phb[:], pre_hi[:], b_hi[:])
            hh = smallp.tile([NO, 1], f32)
            nc.scalar.activation(hh[:], phb[:], AF.Tanh)

            ps_f = ps_misc.tile([NO, 1], f32, tag="gen", name="ps_f")
            nc.tensor.matmul(ps_f[:], h_lo[:NH, :], hl[:], start=True, stop=False)
            nc.tensor.matmul(ps_f[:], h_hi[:NO, :], hh[:], start=False, stop=True)
            fin = smallp.tile([NO, 1], f32)
            nc.scalar.activation(fin[:], ps_f[:], AF.Tanh, bias=phb[:])
            nc.scalar.dma_start(out_d[:], fin[:])

    nc.compile()
    return nc


def _shard_inputs(inputs):
    x = np.ascontiguousarray(inputs["x"], dtype=np.float32)
    xTf = np.zeros((NROW, M), np.float32)
    xTf[:D, :] = x.T
    xTf[D, :] = 1.0                     # bias row
    xT_body = np.ascontiguousarray(
        xTf[:60 * 128].reshape(60, 128, M).transpose(1, 0, 2)).astype(np.float16)
    xT_tail = xTf[60 * 128:].astype(np.float16)

    widths = [min(961, D - 961 * c) for c in range(NCORES)]
    offs = [961 * c for c in range(NCORES)]

    # host-side Bayesian sample, only the slices the graph ever reads
    Ws_cols = (inputs["W_mu"][:, D:N] + inputs["W_sigma"][:, D:N]
               * inputs["eps_w"][:, D:N]).astype(np.float32)       # [N, 130]
    h_full = np.ascontiguousarray(
        Ws_cols[D:N, NH:NH + NO], dtype=np.float32)                # [130, 2]
    b_full = (inputs["bias_mu"][D:N] + inputs["bias_sigma"][D:N]
              * inputs["eps_b"][D:N]).astype(np.float32)           # [130]

    in_maps = []
    for c in range(NCORES):
        off, w = offs[c], widths[c]
        im = {"xT": xT_body, "xTt": xT_tail, "h": h_full, "b": b_full}
        for mat, Wn, bn in (("q", "Wq", "bq"), ("k", "Wk", "bk"), ("v", "Wv", "bv")):
            Wt = np.zeros((NROW, JW), np.float32)
            Wt[:D, :w] = inputs[Wn][off:off + w, :].T
            Wt[D, :w] = inputs[bn][off:off + w]
            im[f"{mat}T"] = np.ascontiguousarray(
                Wt[:60 * 128].reshape(60, 128, JW).transpose(1, 0, 2)
            ).astype(np.float16)
            im[f"{mat}Tt"] = Wt[60 * 128:].astype(np.float16)
        g = np.zeros((8 * 128, NH + NO), np.float32)
        g[:w, :] = Ws_cols[off:off + w, :]
        im["g"] = np.ascontiguousarray(
            g.reshape(8, 128, NH + NO).transpose(1, 0, 2)).astype(np.float16)
        in_maps.append(im)
    return in_maps


def _run(inputs, trace=False):
    global _COMPILED
    from concourse.bass_utils import run_bass_kernel_spmd

    if _COMPILED is None:
        _COMPILED = _build_program()
    in_maps = _shard_inputs(inputs)
    res = run_bass_kernel_spmd(
        _COMPILED, in_maps, core_ids=list(range(NCORES)), trace=trace)
    out = np.asarray(res.results[0]["out"], dtype=np.float32).reshape(NO)
    return out, res


def kernel(**inputs):
    out, _ = _run(inputs, trace=False)
    return out
